# revision 37
# baseline (speedup 1.0000x reference)
"""Trainium2 Bass kernel for nn_MixedLinear_KV (moe_routing, memory-bound).

Math: the reference computes
    x_mix = sum_m coef_a[m] * fake_quant(x, a_scales[m], AB[m])
    w_mix = sum_{i,j,n} coef_w[i,j,n] * fake_quant(pad_ij(W), w_scales[n], WB[n])
    b_mix = sum_{i,j} coef_b[i,j] * pad_ij(b)
    out   = x_mix @ w_mix.T + b_mix

With the benchmark inputs (a_scales == 1, x ~ N(0,1) so |x| < 7.5 always,
verified at runtime), both activation fake-quants reduce to rint(x), so
    x_mix = (coef_a[0] + coef_a[1]) * rint(x) = s * rint(x)
and therefore out = rint(x) @ (s * w_mix).T + b_mix = q @ W + b.

Device-side design (per core, data-parallel over the 8 batches):
  - q = rint(x) is computed on HOST and shipped as fp8e4 (exact: small
    integers), cutting the input stream from 16 MiB fp32 to 4 MiB.
  - Columns of W are PERMUTED by predicted fp8 quantization error: the
    256 best columns run as e4m3 (x lambda) with DoubleRow fp8 matmuls
    (2 K-planes per instruction, ~1.8x the bf16 MAC rate); the 256
    worst run in fp16 at the bf16 rate. Empirically the hybrid lands
    at ~6e-3 rel-max error vs the 2e-2 gate (pure fp8 is 2.5e-2: too
    big; pure fp16 is 4e-4 but ~1.3x slower).
  - Output tiles are [128 outs, 512 toks]: fp8-half tiles drain through
    the scalar engine (Identity: psum * (1/lambda) + bias, both
    per-partition APs), fp16-half tiles through the vector engine
    (psum + bias_broadcast), so the two halves drain in parallel.
  - A dozen zero-weight warmup matmuls run during the q DMA head so the
    PE clock is fully ramped when real data lands.
  - Output leaves as outT [512, 4096] fp16 (4 MiB); the host transposes
    and un-permutes the columns.
"""

import sys

sys.path.insert(0, "/opt/trn_rl_repo")

import json

import ml_dtypes
import numpy as np

import concourse.bass as bass
import concourse.mybir as mybir
from concourse import tile
from concourse.bass_utils import run_bass_kernel_spmd

# Problem constants (hardcoded per task contract)
B, S, D_IN, D_OUT = 8, 4096, 1024, 512
HS = [512, 768, 1024]
NH = [8, 12, 16]
NKV = 4
AB = [4, 8]
WB = [4, 8]
N_CORES = 8
KC = D_IN // 128  # 8 contraction chunks
KP = KC // 2  # 4 DoubleRow chunk pairs
N8 = 256  # columns computed in fp8 DoubleRow
N16 = D_OUT - N8  # columns computed in fp16
OC8 = N8 // 128  # 2
OC16 = N16 // 128  # 2
OC = OC8 + OC16
TBG = 4  # t super-blocks
TB_PER_G = 2  # psum tiles per (oc, super-block)
TS = S // (TBG * TB_PER_G)  # 512 tokens per psum tile
TG = TB_PER_G * TS  # 1024 tokens per super-block
F8 = ml_dtypes.float8_e4m3  # TRN FP8_EXP4 (max +-240) == ml_dtypes e4m3
N_WARMUP_MM = 19
S_BY_BLOCK = (KP, KP, 3, 2)  # fp8 kc-pairs per output block (rest fp16)


def _split_multi_waits(bir_bytes: bytes) -> bytes:
    """This container's walrus supports only one sem-wait per instruction;
    hoist extra waits onto preceding NoOps on the same engine."""
    bir = json.loads(bir_bytes)
    for fn in bir["functions"]:
        for bb in fn["blocks"]:
            new_insts = []
            for inst in bb["instructions"]:
                si = inst.get("sync_info") or {}
                ow = si.get("on_wait") or []
                if len(ow) > 1:
                    for k, w in enumerate(ow[:-1]):
                        new_insts.append(
                            {
                                "debug": inst.get("debug", 0),
                                "engine": inst["engine"],
                                "ins": [],
                                "outs": [],
                                "name": f"{inst['name']}_wsplit{k}",
                                "opcode": "NoOp",
                                "sync_info": {"on_wait": [w]},
                            }
                        )
                    si["on_wait"] = [ow[-1]]
                new_insts.append(inst)
            bb["instructions"] = new_insts
    return json.dumps(bir).encode()


def _host_fold_weights(weight, bias, mix_weights, a_scales, w_scales):
    """Mirror the reference's fp32 weight mixture exactly; return
    (wt_f16 [1024,512], b_mix_f32 [512], w_mix [512,1024])."""
    w32 = np.asarray(weight, np.float32)
    b32 = np.asarray(bias, np.float32)
    mw = np.asarray(mix_weights, np.float32).reshape(3, 3, 2, 2)
    w_sc = np.asarray(w_scales, np.float32)

    coef_a = mw.sum(axis=(0, 1, 3))  # [2]
    coef_w = mw.sum(axis=2)  # [3,3,2]
    coef_b = mw.sum(axis=(2, 3))  # [3,3]

    w_mix = np.zeros((D_OUT, D_IN), np.float32)
    b_mix = np.zeros((D_OUT,), np.float32)
    for i, h in enumerate(HS):
        for j, nh in enumerate(NH):
            out_dim = NKV * (h // nh)
            w_pad = np.zeros((D_OUT, D_IN), np.float32)
            w_pad[:out_dim, :h] = w32[:out_dim, :h]
            b_pad = np.zeros((D_OUT,), np.float32)
            b_pad[:out_dim] = b32[:out_dim]
            for n, wb in enumerate(WB):
                qn, qp = -(2 ** (wb - 1)), 2 ** (wb - 1) - 1
                xs = w_pad / w_sc[n]
                xc = np.clip(xs, np.float32(qn), np.float32(qp))
                fq = np.rint(xc) * w_sc[n]
                w_mix = w_mix + coef_w[i, j, n] * fq
            b_mix = b_mix + coef_b[i, j] * b_pad

    s = np.float64(coef_a[0]) + np.float64(coef_a[1])
    w_eff = s * w_mix.astype(np.float64)  # [512, 1024]
    wt_f16 = np.ascontiguousarray(w_eff.T).astype(np.float16)  # [1024, 512]
    return wt_f16, b_mix, w_mix


def _build_nc(pairs8_by_block):
    """pairs8_by_block: for each of the OC output blocks, the tuple of
    kc-pair indices computed in fp8 DoubleRow (the rest in fp16)."""
    f32, f16, f8 = mybir.dt.float32, mybir.dt.float16, mybir.dt.float8e4
    nc = bass.Bass("TRN2", target_bir_lowering=False, debug=False)

    q_d = nc.dram_tensor("q", [D_IN, S], f8, kind="ExternalInput").ap()
    w8_d = nc.dram_tensor("w8", [128, KP, 2, D_OUT], f8, kind="ExternalInput").ap()
    w16_d = nc.dram_tensor("w16", [128, KC, N16], f16, kind="ExternalInput").ap()
    bc_d = nc.dram_tensor("bc", [128, OC], f32, kind="ExternalInput").ap()
    sc_d = nc.dram_tensor("sc", [128, OC], f32, kind="ExternalInput").ap()
    br_d = nc.dram_tensor("br", [128, OC, TS], f32, kind="ExternalInput").ap()
    out_d = nc.dram_tensor("out", [D_OUT, S], f16, kind="ExternalOutput").ap()

    with tile.TileContext(nc) as tc:
        with (
            tc.tile_pool(name="const", bufs=1) as cpool,
            tc.tile_pool(name="qp", bufs=TBG) as qpool,
            tc.tile_pool(name="op", bufs=8) as opool,
            tc.tile_pool(name="ps", bufs=8, space="PSUM") as pspool,
        ):
            # PE warmup: zeroed fp16 dummy matmuls with no DMA deps keep the
            # clock ramping while the first q blocks stream in. The dummy
            # psum tile comes from the regular pool so it recycles.
            wdum = cpool.tile([128, 128], f16)
            nc.vector.memset(wdum[:], 0.0)
            psdum = pspool.tile([128, TS], f32, tag="ps", name="psdum")
            for _ in range(N_WARMUP_MM):
                nc.tensor.matmul(
                    psdum[:, :128],
                    lhsT=wdum[:],
                    rhs=wdum[:],
                    start=True,
                    stop=True,
                )

            # DMA order is transfer-bandwidth aware: only the slabs the first
            # matmuls need go ahead of g0's q; everything else queues after
            w8_sb = cpool.tile([128, KP, 2, D_OUT], f8)
            w16_sb = cpool.tile([128, KC, N16], f16)
            bc_sb = cpool.tile([128, OC], f32)
            sc_sb = cpool.tile([128, OC], f32)
            br_sb = cpool.tile([128, OC, TS], f32)
            q_dr = q_d.rearrange("(kc p) t -> p kc t", p=128)  # [128, 8, 4096]
            q_sb = {}

            def w8_dma(p, eng):
                eng.dma_start(out=w8_sb[:, p, :, :], in_=w8_d[:, p, :, :])

            def q_dma(g):
                qt = qpool.tile([128, KC, TG], f8, tag="q", name=f"q_{g}")
                for p in range(KP):
                    eng = nc.sync if p % 2 == 0 else nc.scalar
                    eng.dma_start(
                        out=qt[:, 2 * p : 2 * p + 2, :],
                        in_=q_dr[:, 2 * p : 2 * p + 2, g * TG : (g + 1) * TG],
                    )
                q_sb[g] = qt

            def w16_dma(oc, eng):
                # only the chunks the block's fp16 matmuls actually read
                c0 = (oc - OC8) * 128
                for p in range(KP):
                    if p not in pairs8_by_block[oc]:
                        eng.dma_start(
                            out=w16_sb[:, 2 * p : 2 * p + 2, c0 : c0 + 128],
                            in_=w16_d[:, 2 * p : 2 * p + 2, c0 : c0 + 128],
                        )

            w8_dma(0, nc.sync)
            w8_dma(1, nc.scalar)
            q_dma(0)
            w8_dma(2, nc.sync)
            w8_dma(3, nc.scalar)
            nc.scalar.dma_start(out=bc_sb[:], in_=bc_d[:])
            nc.scalar.dma_start(out=sc_sb[:], in_=sc_d[:])
            w16_dma(OC8, nc.scalar)
            w16_dma(OC8 + 1, nc.scalar)
            nc.sync.dma_start(out=br_sb[:], in_=br_d[:])
            for g in range(1, TBG):
                q_dma(g)

            def drain(g, oc, ps_t, tb):
                # psums are uniformly lambda-scaled. tb0 drains on the scalar
                # engine (x 1/lambda + bias -> unscaled f16); tb1 on the
                # vector engine (+ lambda*bias -> lambda-scaled f16, host
                # rescales). Per-half out DMAs on sync / gpsimd.
                o_sb = opool.tile([128, TS], f16, tag="o", name=f"o_{g}_{oc}_{tb}")
                if tb == 0:
                    nc.scalar.activation(
                        o_sb[:],
                        ps_t[:],
                        mybir.ActivationFunctionType.Identity,
                        bias=bc_sb[:, oc : oc + 1],
                        scale=sc_sb[:, oc : oc + 1],
                    )
                    oeng = nc.sync
                else:
                    nc.vector.tensor_add(o_sb[:], ps_t[:], br_sb[:, oc, :])
                    # HWDGE for the final group so the drain tail is short
                    # final group drains on the by-then-idle scalar HWDGE so
                    # the two tail transfers leave on parallel queues
                    oeng = nc.gpsimd if g < TBG - 1 else nc.scalar
                t0 = g * TG + tb * TS
                oeng.dma_start(
                    out=out_d[oc * 128 : (oc + 1) * 128, t0 : t0 + TS],
                    in_=o_sb[:],
                )

            def emit_block(g, oc):
                """One [128, TG] output block: fp8 DoubleRow on the chosen
                kc-pairs, fp16 on the remaining chunks, all into one psum
                group per tile, walking pairs in DMA-arrival order."""
                fp8_pairs = set(pairs8_by_block[oc])
                ps = [
                    pspool.tile([128, TS], f32, tag="ps", name=f"ps_{g}_{oc}_{tb}")
                    for tb in range(TB_PER_G)
                ]
                n_mm = KP + sum(1 for p in range(KP) if p not in fp8_pairs)
                i_mm = 0
                for p in range(KP):
                    if p in fp8_pairs:
                        lhsT = w8_sb[:, p, :, oc * 128 : (oc + 1) * 128]
                        for tb in range(TB_PER_G):
                            nc.tensor.matmul(
                                ps[tb][:],
                                lhsT=lhsT,
                                rhs=q_sb[g][
                                    :, 2 * p : 2 * p + 2, tb * TS : (tb + 1) * TS
                                ],
                                start=(i_mm == 0),
                                stop=(i_mm == n_mm - 1),
                                perf_mode=mybir.MatmulPerfMode.DoubleRow,
                            )
                        i_mm += 1
                    else:
                        for kc in (2 * p, 2 * p + 1):
                            lhsT = w16_sb[:, kc, (oc - OC8) * 128 : (oc - OC8 + 1) * 128]
                            for tb in range(TB_PER_G):
                                nc.tensor.matmul(
                                    ps[tb][:],
                                    lhsT=lhsT,
                                    rhs=q_sb[g][:, kc, tb * TS : (tb + 1) * TS],
                                    start=(i_mm == 0),
                                    stop=(i_mm == n_mm - 1),
                                )
                            i_mm += 1
                for tb in range(TB_PER_G):
                    drain(g, oc, ps[tb], tb)

            for g in range(TBG):
                for oc in range(OC):
                    emit_block(g, oc)

    orig = nc.to_json_bytes
    nc.to_json_bytes = lambda: _split_multi_waits(orig())
    return nc


_NC_CACHE = {}


def _fq32(x, scale, bits):
    """fp32 fake_quant forward value, matching the reference bitwise."""
    qn, qp = -(2 ** (bits - 1)), 2 ** (bits - 1) - 1
    xs = (np.asarray(x, np.float32) / np.float32(scale)).astype(np.float32)
    xc = np.clip(xs, np.float32(qn), np.float32(qp))
    return (np.rint(xc) * np.float32(scale)).astype(np.float32)


def _x_mix_ref(x, mix_weights, a_scales):
    """The reference's activation mixture, in fp32."""
    mw = np.asarray(mix_weights, np.float32).reshape(3, 3, 2, 2)
    coef_a = mw.sum(axis=(0, 1, 3))
    xm = coef_a[0] * _fq32(x, a_scales[0], AB[0])
    return (xm + coef_a[1] * _fq32(x, a_scales[1], AB[1])).astype(np.float32)


def _split_weights(wt_f16):
    """Column-permute W [1024, 512] by predicted fp8 error. Best N8 columns
    run fully in fp8 DoubleRow; the other blocks run fp8 on their 2
    lowest-error kc-pairs and (lambda-scaled) fp16 on the rest. Returns host
    arrays, the per-block fp8-pair tuples, perm, lam, and the effective
    fp32 device weight for outlier patching."""
    W = np.asarray(wt_f16, np.float32)  # [1024, 512]
    lam = np.float32(2.0 ** np.floor(np.log2(224.0 / max(np.abs(W).max(), 1e-30))))
    W8f = np.asarray(W * lam, F8).astype(np.float32)  # e4m3(lam*W) decoded
    E = W8f / lam - W
    sigma = np.sqrt((E * E).sum(axis=0))
    perm = np.argsort(sigma, kind="stable").astype(np.int64)

    Wp = W[:, perm]  # permuted columns
    Wp8 = np.asarray(Wp * lam, F8)  # [1024, 512] e4m3, all columns
    w8_dr = np.ascontiguousarray(Wp8.reshape(KP, 2, 128, D_OUT).transpose(2, 0, 1, 3))
    W16 = (Wp[:, N8:] * lam).astype(np.float16)  # lambda-scaled fp16
    w16_dr = np.ascontiguousarray(W16.reshape(KC, 128, N16).transpose(1, 0, 2))

    # per-block fp8 kc-pairs: full-fp8 blocks use all; staircase blocks the
    # S_BY_BLOCK pairs with least quantization-error energy over the block
    Ep = (E[:, perm] ** 2).reshape(KP, 256, OC, 128).sum(axis=(1, 3))  # [KP, OC]
    pairs8 = []
    for oc in range(OC):
        s = S_BY_BLOCK[oc]
        if s >= KP:
            pairs8.append(tuple(range(KP)))
        else:
            pairs8.append(tuple(sorted(np.argsort(Ep[:, oc])[:s].tolist())))

    w_dev32 = np.empty((D_IN, D_OUT), np.float32)
    for oc in range(OC):
        cols = perm[oc * 128 : (oc + 1) * 128]
        for p in range(KP):
            ks = slice(256 * p, 256 * (p + 1))
            if p in pairs8[oc]:
                w_dev32[ks, cols] = (
                    Wp8[ks, oc * 128 : (oc + 1) * 128].astype(np.float32) / lam
                )
            else:
                w_dev32[ks, cols] = (
                    W16[ks, (oc - OC8) * 128 : (oc - OC8 + 1) * 128].astype(
                        np.float32
                    )
                    / lam
                )
    return w8_dr, w16_dr, lam, tuple(pairs8), perm, w_dev32


def _prepare_in_maps(x, wt_f16, b_mix):
    """Host-side shard prep. Returns (in_maps, pairs8, lam, q8, perm, w_dev32)."""
    q8 = np.clip(np.rint(np.asarray(x, np.float32)), -240.0, 240.0).astype(F8)
    w8_dr, w16_dr, lam, pairs8, perm, w_dev32 = _split_weights(wt_f16)
    bp = np.asarray(b_mix, np.float32)[perm]  # permuted bias
    bc = np.ascontiguousarray(bp.reshape(OC, 128).T).astype(np.float32)
    sc = np.full((128, OC), 1.0 / lam, np.float32)
    br = np.ascontiguousarray(
        np.broadcast_to((lam * bp).reshape(OC, 128).T[:, :, None], (128, OC, TS))
    ).astype(np.float32)
    shared = {"w8": w8_dr, "w16": w16_dr, "bc": bc, "sc": sc, "br": br}
    in_maps = [
        {"q": np.ascontiguousarray(q8[b].T), **shared} for b in range(N_CORES)
    ]
    return in_maps, pairs8, lam, q8, perm, w_dev32


def kernel(x, weight, bias, mix_weights, a_scales, w_scales):
    global _NC_CACHE
    x = np.asarray(x, np.float32)
    assert x.shape == (B, S, D_IN)
    a_sc = np.asarray(a_scales, np.float32)

    wt_f16, b_mix, w_mix = _host_fold_weights(
        weight, bias, mix_weights, a_scales, w_scales
    )

    if not np.all(a_sc == np.float32(1.0)):
        # General-scale fallback (benchmark inputs always have a_scales == 1):
        # compute the reference mixture on host in fp32.
        x_mix = _x_mix_ref(x, mix_weights, a_scales)
        return (np.einsum("bsi,oi->bso", x_mix, w_mix) + b_mix).astype(np.float32)

    in_maps, pairs8, lam, q8, perm, w_dev32 = _prepare_in_maps(x, wt_f16, b_mix)
    if pairs8 not in _NC_CACHE:
        _NC_CACHE[pairs8] = _build_nc(pairs8)
    nc = _NC_CACHE[pairs8]

    try:
        res = run_bass_kernel_spmd(nc, in_maps, list(range(N_CORES)))
    except Exception:
        # one retry for transient device errors
        res = run_bass_kernel_spmd(nc, in_maps, list(range(N_CORES)))

    # vector-engine-drained halves (t in [g*TG+TS, (g+1)*TG)) come back
    # lambda-scaled
    out = np.empty((B, S, D_OUT), np.float32)
    overflow = False
    for b in range(N_CORES):
        dev = res.results[b]["out"]  # [512, 4096] f16, permuted rows
        dev32 = dev.astype(np.float32).reshape(D_OUT, TBG, TB_PER_G, TS)
        overflow = overflow or bool(np.isinf(dev[:]).any())
        dev32[:, :, 1, :] *= np.float32(1.0 / lam)
        out[b][:, perm] = dev32.reshape(D_OUT, S).T
    if overflow:
        # lambda-scaled fp16 overflowed (pathological inputs): exact host path
        x_mix = _x_mix_ref(x, mix_weights, a_scales)
        return (np.einsum("bsi,oi->bso", x_mix, w_mix) + b_mix).astype(np.float32)

    # Exact-intent host patch for |x| >= 7.49, where rint(x) differs from the
    # reference's clipped fake-quants (x ~ N(0,1) in the benchmark: never
    # triggers; keeps kernel() correct for arbitrary inputs).
    idx = np.argwhere(np.abs(x) >= 7.49)
    if len(idx):
        for b, t, i in idx:
            xv = x[b, t, i]
            ref_xmix = _x_mix_ref(xv, mix_weights, a_sc)
            dev_q = np.float32(q8[b, t, i])  # what the device multiplied
            out[b, t, :] += ref_xmix * w_mix[:, i] - dev_q * w_dev32[i, :]
    return out
